# revision 10
# baseline (speedup 1.0000x reference)
"""LIF neuron kernel for Trainium2, 8-core SPMD (batch-sharded).

Reference semantics per timestep t (fp32, TAU=0.5):
    u   = 0.5*m + x_t          # leaky integrate
    s   = (u >= thresh)        # fire (output, 1.0/0.0)
    m'  = u * (u < thresh)     # hard reset

Bit-exactness: 0.5*m is exact in fp32 (power of two), so computing
u = (m mult 0.5) add x_t with one rounding matches the reference's
fl(fl(0.5*m) + x) exactly.  The compare and the multiply-by-{0,1} are
exact, so the kernel reproduces the fp32 reference bit-for-bit.

Per-core layout: batches 8c..8c+7.  Lanes (b_local, n) are mapped to
SBUF as partition p = b_local*16 + (n // 256), free f = n % 256, so a
timestep is one [128, 256] tile.  Host pre-transposes x to [T, 128, 256]
per core so every DMA is a clean strided AP.

Engine split per timestep:
  DVE:    u = scalar_tensor_tensor(m, 0.5, x_t; mult, add)
          m = custom_dve LIF_RESET(u, th)  (select(u < th, u, 0), 1 uop)
  GPSIMD: s = tensor_tensor(u, th, is_ge)  -> spike output tile
  SP:     HWDGE DMAs, 10-timestep chunks, double buffered.
"""

import numpy as np

import concourse.bass as bass
import concourse.bacc as bacc
import concourse.mybir as mybir
from concourse import tile
from concourse.bass_utils import run_bass_kernel_spmd

B, T, N = 64, 100, 4096
NCORES = 8
BL = B // NCORES          # local batches per core
C = 16                    # feature chunks -> partitions
F = N // C                # 256 features per chunk
P = BL * C                # 128 partitions
TCHK = 10                 # timesteps per DMA chunk
NCHK = T // TCHK

_F32 = mybir.dt.float32
_ALU = mybir.AluOpType

# ---------------------------------------------------------------- custom op --

_LIF_OP = None


def _register_lif_op():
    """Register the fused reset op select(u < th, u, 0) at runtime."""
    global _LIF_OP
    if _LIF_OP is not None:
        return _LIF_OP
    from concourse.dve_spec import C2, Spec, Src0, Src1, Zero, select, lower
    from concourse.dve_uop import DveOpSpec
    from concourse import dve_ops as dom

    name = "LIF_RESET_ANT"
    for op in dom.OPS:
        if op.name == name:
            _LIF_OP = op
            return op

    # h' = 0.5 * u * (u < th): fused reset + leak (imm2 = 0.5 at call site).
    spec = Spec(
        body=select(Src0 < Src1, Src0, Zero) * C2,
        reference=lambda in0, in1, s0, s1, imm2: (
            np.where(in0 < in1, in0, np.float32(0.0)) * np.float32(imm2)
        ).astype(np.float32),
    )
    shas = {}
    for ver in ("v3", "v4"):
        try:
            tmp = DveOpSpec(name=name, opcode=None, uops=lower(spec, ver=ver), rd1_en=True)
            shas[ver] = tmp.sha(ver)
        except Exception:
            pass
    op = dom.DveOp(name, spec, subdim=False, uops_sha=shas)
    dom.OPS.append(op)
    dom._SUB_OPCODE_FOR_NAME[name] = dom._CUSTOM_DVE_ROW_BASE + len(dom.OPS) - 1
    dom.CUSTOM_DVE_SPECS[name] = spec
    _LIF_OP = op
    return op


# ------------------------------------------------------------------ program --

_NC_CACHE = {}


def _build_bass():
    if "nc" in _NC_CACHE:
        return _NC_CACHE["nc"]
    lif_op = _register_lif_op()

    nc = bacc.Bacc("TRN2", name="lif_kernel")
    xt = nc.dram_tensor("xt", [T, P, F], _F32, kind="ExternalInput")
    tht = nc.dram_tensor("tht", [P, F], _F32, kind="ExternalInput")
    spk = nc.dram_tensor("spk", [T, P, F], _F32, kind="ExternalOutput")

    with tile.TileContext(nc) as tc:
        with (
            tc.tile_pool(name="const", bufs=1) as cpool,
            tc.tile_pool(name="xin", bufs=3) as xpool,
            tc.tile_pool(name="sout", bufs=3) as spool,
            tc.tile_pool(name="uw", bufs=3) as upool,
        ):
            th_t = cpool.tile([P, F], _F32)
            nc.sync.dma_start(th_t[:], tht[:])
            m = cpool.tile([P, F], _F32)
            nc.vector.memset(m[:], 0.0)

            for k in range(NCHK):
                x_tile = xpool.tile([P, TCHK, F], _F32)
                nc.sync.dma_start(
                    x_tile[:], xt[k * TCHK:(k + 1) * TCHK].rearrange("t p f -> p t f")
                )
                s_tile = spool.tile([P, TCHK, F], _F32)
                for tl in range(TCHK):
                    u = upool.tile([P, F], _F32, tag="u")
                    # u = h + x_t  (h tracks m/2, so this is 0.5*m + x_t)
                    nc.vector.tensor_tensor(
                        u[:], m[:], x_tile[:, tl, :], _ALU.add
                    )
                    # Spike path, lane-split (Pool has no compare ALU ops, so
                    # it uses v = u - th, sign-exact, then TS is_ge(v, 0);
                    # DVE takes the tail columns with a direct is_ge).
                    FP = 192
                    v = upool.tile([P, FP], _F32, tag="v")
                    nc.gpsimd.tensor_tensor(
                        v[:], u[:, 0:FP], th_t[:, 0:FP], _ALU.subtract
                    )
                    nc.gpsimd.tensor_scalar(
                        out=s_tile[:, tl, 0:FP], in0=v[:], scalar1=0.0,
                        scalar2=None, op0=_ALU.is_ge,
                    )
                    nc.vector.tensor_tensor(
                        s_tile[:, tl, FP:F], u[:, FP:F], th_t[:, FP:F],
                        _ALU.is_ge,
                    )
                    # h' = 0.5 * u * (u < th)
                    nc.vector._custom_dve(
                        lif_op, out=m[:], in0=u[:], in1=th_t[:], imm2=0.5
                    )
                nc.sync.dma_start(
                    spk[k * TCHK:(k + 1) * TCHK].rearrange("t p f -> p t f"), s_tile[:]
                )

    nc.finalize()
    _NC_CACHE["nc"] = nc
    return nc


# -------------------------------------------------------------------- entry --

def _run(x, thresh, trace=False):
    nc = _build_bass()
    x = np.ascontiguousarray(x, dtype=np.float32)
    thresh = np.ascontiguousarray(thresh, dtype=np.float32)
    tht = np.tile(thresh.reshape(C, F), (BL, 1))          # [128, 256]
    in_maps = []
    for c in range(NCORES):
        xc = (
            x[c * BL:(c + 1) * BL]
            .reshape(BL, T, C, F)
            .transpose(1, 0, 2, 3)
            .reshape(T, P, F)
        )
        in_maps.append({"xt": np.ascontiguousarray(xc), "tht": tht})

    res = run_bass_kernel_spmd(
        nc, in_maps, core_ids=list(range(NCORES)), trace=trace
    )
    outs = []
    for c in range(NCORES):
        s = np.asarray(res.results[c]["spk"])              # [T, 128, 256]
        outs.append(
            s.reshape(T, BL, C, F).transpose(1, 0, 2, 3).reshape(BL, T, N)
        )
    return np.concatenate(outs, axis=0), res


def kernel(x, thresh):
    out, _ = _run(x, thresh, trace=False)
    return out
